# revision 16
# baseline (speedup 1.0000x reference)
"""Trainium2 Bass kernel for nn_Cross_Attention (8-core data-parallel over batch).

v3 streaming design:
- SWDGE cast-DMAs load kv/q f32->f16 (no conversion pass).
- conv1 on PE; depthwise 3x3 split between PE (diagonal matmuls on flat
  wrap-around slices + fixups) and DVE (STT chains on a +1-shifted copy),
  per (chunk, 32-row slab).
- k/v produced slab-wise: k slabs DMA-xbar-transposed straight into kdT
  [pix, row, 192]; v slabs stored to DRAM f16 (reloaded in final pass).
- q streamed slab-wise: cast-load -> square-partials -> xbar transpose ->
  Gram accumulation; no full q/qT resident.
- L2 norms folded into softmax logits via S = outer(scale/|q|, 1/|k|)
  (tiny DRAM bounce to turn norm columns into rows).
- proj fused into attn@v: MT = (Wp @ A)^T precompute, one pass over v,
  out stored f16 (host casts to f32).
"""

import os
import sys
from contextlib import ExitStack

sys.path.insert(0, "/opt/trn_rl_repo")

import numpy as np

import concourse.bass as bass
import concourse.tile as tile
from concourse import bacc, mybir
from concourse.bass_utils import run_bass_kernel_spmd
from concourse.bass_interp import get_hw_module

F32 = mybir.dt.float32
F16 = mybir.dt.float16
MULT = mybir.AluOpType.mult
ADD = mybir.AluOpType.add
BYPASS = mybir.AluOpType.bypass
AX = mybir.AxisListType.X
AF = mybir.ActivationFunctionType

C = 192
C2 = 384
HEADS = 8
CD = C // HEADS
W = 128
H = int(os.environ.get("BASS_CA_H", "128"))
HWTOT = H * W
SLAB_R = 32
NS = H // SLAB_R
SLW = SLAB_R * W                 # 4096 pixels per slab
# kvf rows: 0 zero, 1 top-boundary, 2..33 interior, 34 bottom-boundary, 35 zero
KVF_R = SLAB_R + 4
PE_T = int(os.environ.get("BASS_CA_PET", "5"))   # 4-row tiles per slab on PE
assert 1 <= PE_T <= 8
EPS = 1e-12

TAPS = [(0, 0)] + [(dr, dc) for dr in (-1, 0, 1) for dc in (-1, 0, 1)
                   if not (dr == 0 and dc == 0)]


def emit_slab(tc, io, sb, mc, s):
    """conv1 + depthwise for chunk mc, slab s. Output lands in a rotating
    slab tile: mc0 -> kd_a rows, mc1 -> [kd_b ; vd_lo], mc2 -> vd_hi."""
    nc = tc.nc
    r0img = s * SLAB_R
    mcs = slice(mc * 128, (mc + 1) * 128)
    ssl = slice(s * SLW, (s + 1) * SLW)

    kvf = sb["kvfp"].tile([128, KVF_R * W], F16, tag="kvf", name="kvf")
    kvf3 = kvf[:].rearrange("p (r c) -> p r c", c=W)
    ds = sb["dsp"].tile([128, SLW], F16, tag="ds", name="ds")
    ds3 = ds[:].rearrange("p (r c) -> p r c", c=W)

    # ---- conv1 interior rows (kvf rows 2..33): 4 psum pairs of 8 rows ----
    for j in range(SLAB_R // 8):
        ps = sb["psc"].tile([128, 1024], F32, tag="psc", name="ps")
        for h in range(2):
            pix = (r0img + 8 * j + 4 * h) * W
            psl = ps[:, h * 512:(h + 1) * 512]
            nc.tensor.matmul(psl, sb["w1ta"][:, mcs], io["kv16a"][:, pix:pix + 512],
                             start=True, stop=False)
            nc.tensor.matmul(psl, sb["w1tb"][:, mcs], io["kv16b"][:, pix:pix + 512],
                             start=False, stop=True)
        nc.scalar.copy(kvf[:, (2 + 8 * j) * W:(10 + 8 * j) * W], ps[:])

    # ---- boundary rows 1 (img r0img-1) and 34 (img r0img+32); 0/35 zero ----
    nc.vector.memset(kvf3[:, 0, :], 0.0)
    nc.vector.memset(kvf3[:, KVF_R - 1, :], 0.0)
    needb = [(krow, img) for krow, img in
             ((1, r0img - 1), (SLAB_R + 2, r0img + SLAB_R)) if 0 <= img < H]
    for krow, img in ((1, r0img - 1), (SLAB_R + 2, r0img + SLAB_R)):
        if not (0 <= img < H):
            nc.vector.memset(kvf3[:, krow, :], 0.0)
    if needb:
        bps = sb["psc"].tile([128, 1024], F32, tag="psc", name="bps")
        for bi, (krow, img) in enumerate(needb):
            psl = bps[:, bi * 128:(bi + 1) * 128]
            pix = img * W
            nc.tensor.matmul(psl, sb["w1ta"][:, mcs], io["kv16a"][:, pix:pix + 128],
                             start=True, stop=False)
            nc.tensor.matmul(psl, sb["w1tb"][:, mcs], io["kv16b"][:, pix:pix + 128],
                             start=False, stop=True)
            nc.scalar.copy(kvf3[:, krow, :], psl)

    # ---- PE depthwise tiles (flat wrap-around slices, fixups after) ----
    for t in range(PE_T):
        ps = sb["psd"].tile([128, 512], F32, tag="psd", name="psd")
        rk = 2 + 4 * t
        for ti, (dr, dc) in enumerate(TAPS):
            wi = (dr + 1) * 3 + (dc + 1)
            lw = sb["w2sb"][:, mc * 9 + wi, :]
            base = (rk + dr) * W + dc
            nc.tensor.matmul(ps[:], lw, kvf[:, base:base + 512],
                             start=(ti == 0), stop=(ti == 8))
        nc.scalar.copy(ds[:, t * 512:(t + 1) * 512], ps[:])
    npe = 4 * PE_T
    for (dr, dc) in TAPS:
        if dc == 0:
            continue
        wi = (dr + 1) * 3 + (dc + 1)
        wnap = sb["w2vn"][:, mc * 9 + wi: mc * 9 + wi + 1]
        if dc == 1:
            badcol, src3 = 127, kvf3[:, 3 + dr: 3 + dr + npe, 0:1]
        else:
            badcol, src3 = 0, kvf3[:, 1 + dr: 1 + dr + npe, 127:128]
        nc.vector.scalar_tensor_tensor(
            out=ds3[:, 0:npe, badcol:badcol + 1], in0=src3, scalar=wnap,
            in1=ds3[:, 0:npe, badcol:badcol + 1], op0=MULT, op1=ADD)

    # ---- DVE depthwise rows 4*PE_T..31 ----
    nr = SLAB_R - 4 * PE_T
    if nr > 0:
        fd = nr * W
        rk = 2 + 4 * PE_T
        acc = [sb["accp"].tile([128, fd], F16, tag=f"acc{i}", name=f"acc{i}")
               for i in range(2)]
        tmp = sb["accp"].tile([128, fd], F16, tag="tmp", name="tmp")
        dvout = ds[:, npe * W:SLW]
        for ti, (dr, dc) in enumerate(TAPS):
            wi = (dr + 1) * 3 + (dc + 1)
            wap = sb["w2v"][:, mc * 9 + wi: mc * 9 + wi + 1]
            base = (rk + dr) * W + dc
            src = kvf[:, base:base + fd]
            if ti == 0:
                nc.vector.tensor_scalar_mul(acc[1][:], src, wap)
            else:
                out = dvout if ti == 8 else acc[(ti + 1) % 2]
                nc.vector.tensor_scalar_mul(tmp[:], src, wap)
                nc.vector.tensor_tensor(out=out[:], in0=tmp[:],
                                        in1=acc[ti % 2][:], op=ADD)
        for (dr, dc) in TAPS:
            if dc == 0:
                continue
            wi = (dr + 1) * 3 + (dc + 1)
            wnap = sb["w2vn"][:, mc * 9 + wi: mc * 9 + wi + 1]
            if dc == 1:
                badcol, src3 = 127, kvf3[:, rk + dr + 1: rk + dr + 1 + nr, 0:1]
            else:
                badcol, src3 = 0, kvf3[:, rk + dr - 1: rk + dr - 1 + nr, 127:128]
            nc.vector.scalar_tensor_tensor(
                out=ds3[:, npe:SLAB_R, badcol:badcol + 1], in0=src3, scalar=wnap,
                in1=ds3[:, npe:SLAB_R, badcol:badcol + 1], op0=MULT, op1=ADD)

    # ---- consume the slab: transposes to kdT / stores to vdram / norms ----
    kdT, nqk = sb["kdT"], sb["nqk"]
    if mc == 0:
        junk = sb["kvfp"].tile([128, SLW], F16, tag="junk", bufs=1, name="junka")
        nc.scalar.activation(junk[:], ds[:], AF.Square,
                             accum_out=nqk["ka"][:, s:s + 1])
        nc.sync.dma_start_transpose(kdT[:, s * SLAB_R:(s + 1) * SLAB_R, 0:128],
                                    ds[:])
    elif mc == 1:
        junk = sb["kvfp"].tile([128, SLW], F16, tag="junk", bufs=1, name="junkb")
        nc.vector.scalar_tensor_tensor(
            out=junk[0:64, :], in0=ds[0:64, :], scalar=1.0, in1=ds[0:64, :],
            op0=BYPASS, op1=MULT, accum_out=nqk["kb"][:, s:s + 1])
        nc.sync.dma_start_transpose(kdT[:, s * SLAB_R:(s + 1) * SLAB_R, 128:192],
                                    ds[0:64, :])
        nc.scalar.dma_start(io["vdram"][0:64, ssl], ds[64:128, :])
    else:
        nc.scalar.dma_start(io["vdram"][64:C, ssl], ds[:])


def emit_kernel(tc, io):
    nc = tc.nc
    st = ExitStack()
    wp = st.enter_context(tc.tile_pool(name="weights", bufs=1))
    sb = {}

    for nm, src, shape, dt in (
            ("w1ta", io["w1t"][0:128, :], [128, C2], F16),
            ("w1tb", io["w1t"][128:C, :], [64, C2], F16),
            ("w2v", io["w2v"][:], [128, 27], F32),
            ("w2vn", io["w2vn"][:], [128, 27], F32),
            ("wpta", io["wpt"][0:128, :], [128, C], F16),
            ("wptb", io["wpt"][128:C, :], [64, C], F16),
            ("maska", io["mask"][0:128, :], [128, C], F32),
            ("maskb", io["mask"][128:C, :], [64, C], F32),
            ("sca", io["scale192"][0:128, :], [128, 1], F32),
            ("scb", io["scale192"][128:C, :], [64, 1], F32)):
        sb[nm] = wp.tile(shape, dt, name=nm)
        nc.sync.dma_start(sb[nm][:], src)
    sb["w2sb"] = wp.tile([128, 27, 128], F16, name="w2sb")
    nc.sync.dma_start(sb["w2sb"][:], io["w2d"].rearrange("t p c -> p t c"))

    sb["kdT"] = wp.tile([128, H, 192], F16, name="kdT")
    sb["nqk"] = {"ka": wp.tile([128, NS], F32, name="nq_ka"),
                 "kb": wp.tile([64, NS], F32, name="nq_kb"),
                 "qa": wp.tile([128, NS], F32, name="nq_qa"),
                 "qb": wp.tile([64, NS], F32, name="nq_qb")}
    mt1 = wp.tile([64, C], F16, name="mt1")
    mt2 = wp.tile([128, C], F16, name="mt2")

    # ================= phase A: conv1 + depthwise, k/v production =========
    with tc.tile_pool(name="kvp", bufs=1) as kvp, \
         tc.tile_pool(name="kvfp", bufs=2) as kvfp, \
         tc.tile_pool(name="accp", bufs=1) as accp, \
         tc.tile_pool(name="dsp", bufs=4) as dsp, \
         tc.tile_pool(name="psc", bufs=2, space="PSUM") as psc, \
         tc.tile_pool(name="psd", bufs=3, space="PSUM") as psd:
        sb.update({"kvfp": kvfp, "accp": accp, "dsp": dsp,
                   "psc": psc, "psd": psd})
        io["kv16a"] = kvp.tile([128, HWTOT], F16, name="kv16a")
        io["kv16b"] = kvp.tile([64, HWTOT], F16, name="kv16b")
        for s in range(NS):
            ssl = slice(s * SLW, (s + 1) * SLW)
            nc.sync.dma_start(io["kv16a"][:, ssl], io["kv"][0:128, ssl])
            nc.sync.dma_start(io["kv16b"][:, ssl], io["kv"][128:C, ssl])
        for mc in range(3):
            for s in range(NS):
                emit_slab(tc, io, sb, mc, s)

    # ================= phase B: q stream + Gram + softmax + MT ============
    nqk = sb["nqk"]
    with tc.tile_pool(name="qsp", bufs=2) as qsp, \
         tc.tile_pool(name="qtp", bufs=2) as qtp, \
         tc.tile_pool(name="smx", bufs=1) as smx, \
         tc.tile_pool(name="psg", bufs=1, space="PSUM") as psg:
        G0 = psg.tile([128, C], F32, tag="G0", name="G0")
        G1 = psg.tile([64, C], F32, tag="G1", name="G1")
        for s in range(NS):
            ssl = slice(s * SLW, (s + 1) * SLW)
            qsa = qsp.tile([128, SLW], F16, tag="qsa", name="qsa")
            nc.scalar.dma_start(qsa[:], io["q"][0:128, ssl])
            qsb = qsp.tile([64, SLW], F16, tag="qsb", name="qsb")
            nc.scalar.dma_start(qsb[:], io["q"][128:C, ssl])
            jq = qtp.tile([128, SLW], F16, tag="jq", name="jq")
            nc.scalar.activation(jq[:], qsa[:], AF.Square,
                                 accum_out=nqk["qa"][:, s:s + 1])
            nc.vector.scalar_tensor_tensor(
                out=jq[0:64, :], in0=qsb[:], scalar=1.0, in1=qsb[:],
                op0=BYPASS, op1=MULT, accum_out=nqk["qb"][:, s:s + 1])
            qta = qtp.tile([128, SLAB_R, 128], F16, tag="qta", name="qta")
            nc.sync.dma_start_transpose(qta[:], qsa[:])
            qtb = qtp.tile([128, SLAB_R, 64], F16, tag="qtb", name="qtb")
            nc.scalar.dma_start_transpose(qtb[:], qsb[:])
            for t in range(SLAB_R):
                tg = s * SLAB_R + t
                nc.tensor.matmul(G0[:], qta[:, t, :], sb["kdT"][:, tg, :],
                                 start=(tg == 0), stop=(tg == H - 1))
                nc.tensor.matmul(G1[:], qtb[:, t, :], sb["kdT"][:, tg, :],
                                 start=(tg == 0), stop=(tg == H - 1))

        # ---- PE heaters: keep HAM warm through the softmax gap ----
        kdTf = sb["kdT"][:].rearrange("p r c -> p (r c)")
        with tc.tile_pool(name="psh", bufs=1, space="PSUM") as psh:
            heat = psh.tile([128, 512], F32, tag="heat", name="heat")
            for i in range(24):
                nc.tensor.matmul(heat[:], sb["w1ta"][:, 0:128],
                                 kdTf[:, 0:512], start=True, stop=True,
                                 skip_group_check=True)

        # ---- norms -> S = outer(scale/|q|, 1/|k|) via tiny DRAM bounce ----
        sqa = smx.tile([128, 1], F32, name="sqa")
        sqb = smx.tile([64, 1], F32, name="sqb")
        ska = smx.tile([128, 1], F32, name="ska")
        skb = smx.tile([64, 1], F32, name="skb")
        for dst, part, scl in ((sqa, "qa", sb["sca"]), (sqb, "qb", sb["scb"]),
                               (ska, "ka", None), (skb, "kb", None)):
            nc.vector.reduce_sum(dst[:], nqk[part][:], axis=AX)
            nc.scalar.sqrt(dst[:], dst[:])
            nc.vector.tensor_scalar_max(dst[:], dst[:], EPS)
            nc.vector.reciprocal(dst[:], dst[:])
            if scl is not None:
                nc.vector.tensor_tensor(out=dst[:], in0=dst[:], in1=scl[:],
                                        op=MULT)
        nc.sync.dma_start(io["nrm"][0:1, 0:128], sqa[:])
        nc.sync.dma_start(io["nrm"][0:1, 128:C], sqb[:])
        nc.sync.dma_start(io["nrm"][1:2, 0:128], ska[:])
        nc.sync.dma_start(io["nrm"][1:2, 128:C], skb[:])
        sqra = smx.tile([1, 128], F32, name="sqra")
        nc.sync.dma_start(sqra[:], io["nrm"][0:1, 0:128])
        sqrb = smx.tile([1, 64], F32, name="sqrb")
        nc.sync.dma_start(sqrb[:], io["nrm"][0:1, 128:C])
        skr = smx.tile([1, C], F32, name="skr")
        nc.sync.dma_start(skr[:], io["nrm"][1:2, :])

        at = {}
        with tc.tile_pool(name="pss", bufs=1, space="PSUM") as pss:
            S0 = pss.tile([128, C], F32, tag="S0", name="S0")
            S1 = pss.tile([64, C], F32, tag="S1", name="S1")
            nc.tensor.matmul(S0[:], sqra[:], skr[:], start=True, stop=True)
            nc.tensor.matmul(S1[:], sqrb[:], skr[:], start=True, stop=True)
            for nm, G, S, mk, rows in (("a", G0, S0, sb["maska"], 128),
                                       ("b", G1, S1, sb["maskb"], 64)):
                ssb = smx.tile([rows, C], F32, name=f"ssb{nm}")
                nc.scalar.copy(ssb[:], S[:])
                lg = smx.tile([rows, C], F32, name=f"lg{nm}")
                nc.vector.tensor_tensor(out=lg[:], in0=G[:], in1=ssb[:], op=MULT)
                nc.vector.tensor_tensor(out=lg[:], in0=lg[:], in1=mk[:], op=ADD)
                mx = smx.tile([rows, 1], F32, name=f"mx{nm}")
                nc.vector.reduce_max(mx[:], lg[:], axis=AX)
                nc.vector.tensor_scalar_mul(mx[:], mx[:], -1.0)
                ssum = smx.tile([rows, 1], F32, name=f"ss{nm}")
                nc.scalar.activation(lg[:], lg[:], AF.Exp, bias=mx[:],
                                     accum_out=ssum[:])
                nc.vector.reciprocal(ssum[:], ssum[:])
                a16 = smx.tile([rows, C], F16, name=f"a16{nm}")
                nc.vector.tensor_scalar_mul(a16[:], lg[:], ssum[:])
                at[nm] = a16

        with tc.tile_pool(name="psm", bufs=1, space="PSUM") as psm:
            MT0 = psm.tile([128, C], F32, tag="MT0", name="MT0")
            MT1 = psm.tile([64, C], F32, tag="MT1", name="MT1")
            nc.tensor.matmul(MT0[:], at["a"][:, 0:128], sb["wpta"][:],
                             start=True, stop=False)
            nc.tensor.matmul(MT0[:], at["b"][:, 0:128], sb["wptb"][:],
                             start=False, stop=True)
            nc.tensor.matmul(MT1[:], at["a"][:, 128:C], sb["wpta"][:],
                             start=True, stop=False)
            nc.tensor.matmul(MT1[:], at["b"][:, 128:C], sb["wptb"][:],
                             start=False, stop=True)
            nc.vector.tensor_copy(mt1[:], MT0[0:64, :])
            nc.vector.tensor_copy(mt2[0:64, :], MT0[64:128, :])
            nc.vector.tensor_copy(mt2[64:128, :], MT1[:])
        with tc.tile_pool(name="psh2", bufs=1, space="PSUM") as psh2:
            heat2 = psh2.tile([128, 512], F32, tag="heat2", name="heat2")
            for i in range(12):
                nc.tensor.matmul(heat2[:], mt2[:, 0:128],
                                 sb["kdT"][:].rearrange("p r c -> p (r c)")[:, 0:512],
                                 start=True, stop=True, skip_group_check=True)

    # ================= phase C: O = (Wp @ A) @ vd =========================
    NG = HWTOT // 2048
    with tc.tile_pool(name="vst", bufs=3) as vst, \
         tc.tile_pool(name="ost", bufs=3) as ost, \
         tc.tile_pool(name="pso", bufs=2, space="PSUM") as pso:
        for g in range(NG):
            gsl = slice(g * 2048, (g + 1) * 2048)
            vA = vst.tile([64, 2048], F16, tag="vA", name="vA")
            nc.scalar.dma_start(vA[:], io["vdram"][0:64, gsl])
            vB = vst.tile([128, 2048], F16, tag="vB", name="vB")
            nc.scalar.dma_start(vB[:], io["vdram"][64:C, gsl])
            for hp in range(2):
                O0 = pso.tile([128, 1024], F32, tag="O0", name="O0")
                O1 = pso.tile([64, 1024], F32, tag="O1", name="O1")
                for t in range(2):
                    vsl = slice(hp * 1024 + t * 512, hp * 1024 + (t + 1) * 512)
                    osl = slice(t * 512, (t + 1) * 512)
                    nc.tensor.matmul(O0[:, osl], mt1[:, 0:128], vA[:, vsl],
                                     start=True, stop=False)
                    nc.tensor.matmul(O0[:, osl], mt2[:, 0:128], vB[:, vsl],
                                     start=False, stop=True)
                    nc.tensor.matmul(O1[:, osl], mt1[:, 128:C], vA[:, vsl],
                                     start=True, stop=False)
                    nc.tensor.matmul(O1[:, osl], mt2[:, 128:C], vB[:, vsl],
                                     start=False, stop=True)
                oa = ost.tile([128, 1024], F16, tag="oa", name="oa")
                ob = ost.tile([64, 1024], F16, tag="ob", name="ob")
                nc.scalar.copy(oa[:], O0[:])
                nc.vector.tensor_copy(ob[:], O1[:])
                psl = slice(g * 2048 + hp * 1024, g * 2048 + (hp + 1) * 1024)
                nc.sync.dma_start(io["out"][0:128, psl], oa[:])
                nc.sync.dma_start(io["out"][128:C, psl], ob[:])
    st.close()


def build_module():
    nc = bacc.Bacc("TRN2")
    io = {}
    io["kv"] = nc.dram_tensor("kv", [C, HWTOT], F16, kind="ExternalInput").ap()
    io["q"] = nc.dram_tensor("q", [C, HWTOT], F16, kind="ExternalInput").ap()
    io["w1t"] = nc.dram_tensor("w1t", [C, C2], F16, kind="ExternalInput").ap()
    io["w2d"] = nc.dram_tensor("w2d", [27, 128, 128], F16, kind="ExternalInput").ap()
    io["w2v"] = nc.dram_tensor("w2v", [128, 27], F32, kind="ExternalInput").ap()
    io["w2vn"] = nc.dram_tensor("w2vn", [128, 27], F32, kind="ExternalInput").ap()
    io["wpt"] = nc.dram_tensor("wpt", [C, C], F16, kind="ExternalInput").ap()
    io["mask"] = nc.dram_tensor("mask", [C, C], F32, kind="ExternalInput").ap()
    io["scale192"] = nc.dram_tensor("scale192", [C, 1], F32, kind="ExternalInput").ap()
    io["out"] = nc.dram_tensor("out", [C, HWTOT], F16, kind="ExternalOutput").ap()
    io["vdram"] = nc.dram_tensor("vdram", [C, HWTOT], F16).ap()
    io["nrm"] = nc.dram_tensor("nrm", [2, C], F32).ap()
    with tile.TileContext(nc) as tc:
        emit_kernel(tc, io)
    nc.compile()
    return nc


def prep_weights(qkv1_w, qkv2_w, proj_w, scale):
    w1 = np.asarray(qkv1_w).reshape(C2, C)
    w1t = np.ascontiguousarray(w1.T).astype(np.float16)
    w2 = np.asarray(qkv2_w).reshape(C2, 9)
    w2d = np.zeros((27, 128, 128), np.float16)
    for mc in range(3):
        for wi in range(9):
            np.fill_diagonal(w2d[mc * 9 + wi], w2[mc * 128:(mc + 1) * 128, wi])
    w2v = np.zeros((128, 27), np.float32)
    for mc in range(3):
        w2v[:, mc * 9:(mc + 1) * 9] = w2[mc * 128:(mc + 1) * 128, :]
    wpt = np.ascontiguousarray(np.asarray(proj_w).reshape(C, C).T).astype(np.float16)
    mask = np.full((C, C), -1e30, np.float32)
    for h in range(HEADS):
        mask[h * CD:(h + 1) * CD, h * CD:(h + 1) * CD] = 0.0
    scale192 = np.repeat(np.asarray(scale).reshape(HEADS), CD).astype(
        np.float32).reshape(C, 1)
    return {"w1t": w1t, "w2d": w2d, "w2v": w2v, "w2vn": -w2v, "wpt": wpt,
            "mask": mask, "scale192": scale192}


_CACHED = {}


def kernel(kv, q, qkv1_w, qkv2_w, proj_w, scale):
    kv = np.asarray(kv, np.float32)
    q = np.asarray(q, np.float32)
    b = kv.shape[0]
    assert b == 8 and kv.shape[1] == C
    wts = prep_weights(qkv1_w, qkv2_w, proj_w, scale)
    if "nc" not in _CACHED:
        nc = build_module()
        nc.m = get_hw_module(nc.m)
        _CACHED["nc"] = nc
    nc = _CACHED["nc"]
    in_maps = []
    for i in range(b):
        m = {"kv": np.ascontiguousarray(kv[i].reshape(C, HWTOT)).astype(np.float16),
             "q": np.ascontiguousarray(q[i].reshape(C, HWTOT)).astype(np.float16)}
        m.update(wts)
        in_maps.append(m)
    res = run_bass_kernel_spmd(nc, in_maps, core_ids=list(range(8)))
    out = np.stack([res.results[i]["out"].reshape(C, H, W) for i in range(b)])
    return out.astype(np.float32)
